# revision 31
# baseline (speedup 1.0000x reference)
"""Self-contained Trainium2 Bass kernel for causal multi-head attention.

The reference computes softmax(QK^T/sqrt(D)) * tril, renormalized — which is
exactly a causal softmax (the full-row max/normalizer cancel) — followed by
P@V, head concat and an output projection. The time-decay branch is dead code.

Sharding: 16 (batch, head) pairs across 8 cores, 2 heads of one batch per
core. Each core computes partial_out = sum_h (attn_h @ wO_h) for its 2 heads
([2048, 256]); the host adds the 4 partials per batch element.

Per-core layout (all SBUF, fp32):
  XT [d, s]            model-dim-major activations
  QT/KT [e, s]         projections computed transposed so scores come out as
                       S^T[j, i] = KT.T @ QT with no softmax-side transposes
  V [s, e | 1 | 0]     ones column appended -> PV matmul emits row sums free
  PV accumulation in PSUM over causal j chunks; normalization (1/rowsum) is
  folded into the PSUM->SBUF copy; PE transposes flip ret [i,e] -> [e,i] for
  the head-sliced output projection.
"""

from contextlib import ExitStack

import numpy as np

B, S, D, H = 2, 2048, 256, 8
N_CORES = 8
P = 128          # partition size
SG = 512         # query group (i) width
NSG = S // SG    # 4 query groups
NJC = S // P     # 16 key chunks
EC = D // P      # 2 chunks along the head dim e
DC = D // P      # 2 chunks along the model dim d
VW = 258         # V tile width: 256 e cols + ones col + zero pad (even for f32r)

# Matmul operand dtype knobs: "f32" (exact, 4 cyc/row) or "f32r" (1 cyc/row).
import os as _os
ATTN_DT = _os.environ.get("ATTN_DT", "f32")
PROJ_DT = _os.environ.get("PROJ_DT", "f32")

_STATE = {}


def _build_nc():
    import concourse.tile as tile
    from concourse import bacc, mybir

    f32 = mybir.dt.float32
    f32r = mybir.dt.float32r
    attn_dt = f32r if ATTN_DT == "f32r" else f32
    proj_dt = f32r if PROJ_DT == "f32r" else f32

    def cast(ap, kind):
        return ap

    nc = bacc.Bacc("TRN2", target_bir_lowering=False, debug=False,
                   num_devices=N_CORES)

    xt_d = nc.dram_tensor("xt", [D, S], f32, kind="ExternalInput")
    wq_d = nc.dram_tensor("wq", [2, DC, P, D], f32, kind="ExternalInput")
    wk_d = nc.dram_tensor("wk", [2, DC, P, D], f32, kind="ExternalInput")
    wvo_d = nc.dram_tensor("wvo", [2, DC, P, D], f32, kind="ExternalInput")
    mask_d = nc.dram_tensor("mask", [P, 4 * SG], f32, kind="ExternalInput")
    out_d = nc.dram_tensor("out", [S, D], f32, kind="ExternalOutput")

    with tile.TileContext(nc) as tc, ExitStack() as ctx:
        pool = lambda name, bufs, **kw: ctx.enter_context(
            tc.tile_pool(name=name, bufs=bufs, **kw))
        consts = pool("consts", 1)
        xtp = pool("xt", 2)
        stg = pool("stg", 2)
        wts = pool("wts", 8)
        qkp = pool("qk", 8)
        vp = pool("v", 2 * NJC)
        ptp = pool("pt", 4)
        rsp = pool("rs", 8)
        retp = pool("ret", NJC + 8)
        outp = pool("outsb", 4)
        ps_big = pool("ps_big", 4, space="PSUM")
        ps_acc = pool("ps_acc", 4, space="PSUM")

        mask_sb = consts.tile([P, 4 * SG], f32)
        nc.sync.dma_start(out=mask_sb[:], in_=mask_d[:])

        xt_sb = []
        for dc in range(DC):
            t = xtp.tile([P, S], proj_dt, tag="xt")
            if proj_dt == f32:
                nc.sync.dma_start(out=t[:], in_=xt_d[dc * P:(dc + 1) * P, :])
            else:
                raw = stg.tile([P, S], f32, tag="stg", name="xtraw")
                nc.sync.dma_start(out=raw[:], in_=xt_d[dc * P:(dc + 1) * P, :])
                nc.vector.tensor_copy(out=t[:], in_=raw[:])
            xt_sb.append(t)

        w_sb = {}
        for name, dram in (("wq", wq_d), ("wk", wk_d), ("wvo", wvo_d)):
            for h in range(2):
                t = wts.tile([P, DC, D], proj_dt, tag="w")
                if proj_dt == f32:
                    for dc in range(DC):
                        nc.sync.dma_start(out=t[:, dc, :], in_=dram[h, dc])
                else:
                    raw = stg.tile([P, DC, D], f32, tag="wstg", name="wraw")
                    for dc in range(DC):
                        nc.sync.dma_start(out=raw[:, dc, :], in_=dram[h, dc])
                    nc.vector.tensor_copy(out=t[:], in_=raw[:])
                w_sb[name, h] = t

        ret0 = {}  # ic -> head0 normalized output chunk [P, D]

        for h in range(2):
            # --- projections: QT/KT [e, s] and V [s, e | ones] ---
            qt = [qkp.tile([P, S], attn_dt, tag="qk", name="qt")
                  for _ in range(EC)]
            kt = [qkp.tile([P, S], attn_dt, tag="qk", name="kt")
                  for _ in range(EC)]
            for dst, wname in ((qt, "wq"), (kt, "wk")):
                w = w_sb[wname, h]
                for ec in range(EC):
                    for sg in range(NSG):
                        ps = ps_big.tile([P, SG], f32, tag="big")
                        for dc in range(DC):
                            nc.tensor.matmul(
                                ps[:],
                                cast(w[:, dc, ec * P:(ec + 1) * P], "proj"),
                                cast(xt_sb[dc][:, sg * SG:(sg + 1) * SG],
                                     "proj"),
                                start=(dc == 0), stop=(dc == DC - 1))
                        nc.vector.tensor_copy(
                            out=dst[ec][:, sg * SG:(sg + 1) * SG], in_=ps[:])
            v_sb = []
            wv = w_sb["wvo", h]
            for jc in range(NJC):
                ps = ps_acc.tile([P, VW], f32, tag="acc")
                for dc in range(DC):
                    nc.tensor.matmul(
                        ps[:, 0:D],
                        cast(xt_sb[dc][:, jc * P:(jc + 1) * P], "proj"),
                        cast(wv[:, dc, :], "proj"),
                        start=(dc == 0), stop=(dc == DC - 1))
                vt = vp.tile([P, VW], attn_dt, tag="v")
                nc.gpsimd.memset(vt[:, D:D + 1].bitcast(f32), 1.0)
                nc.gpsimd.memset(vt[:, D + 1:VW].bitcast(f32), 0.0)
                nc.vector.tensor_copy(out=vt[:, 0:D], in_=ps[:, 0:D])
                v_sb.append(vt)

            # --- attention: S^T chunks -> exp -> (mask) -> PV accumulate ---
            # Diagonal chunks are trimmed: chunk t only needs i >= t*128, so
            # compute columns [c0, SG) with c0 = min(t,2)*128 (c0 capped so
            # the f32r moving dim stays >= 256) and skip PV sub-blocks ib < t.
            for qo in range(NSG):
                njc = (qo + 1) * 4
                po = [ps_acc.tile([P, VW], f32, tag="acc", name="po") for _ in range(4)]

                def emit_pv(pjc, ppt, t):
                    for ib in range(max(t, 0), 4):
                        nc.tensor.matmul(
                            po[ib][:],
                            cast(ppt[:, ib * P:(ib + 1) * P], "attn"),
                            cast(v_sb[pjc][:], "attn"),
                            start=(pjc == 0), stop=(ib == t))

                pending = []
                for jc in range(njc):
                    t = jc - qo * 4
                    c0 = min(t, 2) * P if t > 0 else 0
                    ps = ps_big.tile([P, SG], f32, tag="big")
                    for ec in range(EC):
                        nc.tensor.matmul(
                            ps[:, c0:SG],
                            cast(kt[ec][:, jc * P:(jc + 1) * P], "attn"),
                            cast(qt[ec][:, qo * SG + c0:(qo + 1) * SG],
                                 "attn"),
                            start=(ec == 0), stop=(ec == EC - 1))
                    pt = ptp.tile([P, SG], f32, tag="pt")
                    nc.scalar.activation(
                        out=pt[:, c0:SG], in_=ps[:, c0:SG],
                        func=mybir.ActivationFunctionType.Exp, scale=1.0 / 16.0)
                    if attn_dt == f32:
                        if t >= 0:
                            nc.vector.tensor_mul(
                                pt[:, c0:SG], pt[:, c0:SG],
                                mask_sb[:, t * SG + c0:(t + 1) * SG])
                        pv_src = pt
                    else:
                        # ACT can't emit f32r; round on DVE (the diagonal
                        # chunks fuse the causal mask into the rounding mul).
                        ptr = ptp.tile([P, SG], attn_dt, tag="ptr", name="ptr")
                        if t >= 0:
                            nc.vector.tensor_mul(
                                ptr[:, c0:SG], pt[:, c0:SG],
                                mask_sb[:, t * SG + c0:(t + 1) * SG])
                        else:
                            nc.vector.tensor_copy(out=ptr[:], in_=pt[:])
                        pv_src = ptr
                    pending.append((jc, pv_src, t))
                    if len(pending) > 3:
                        pjc, ppt, pp_t = pending.pop(0)
                        emit_pv(pjc, ppt, pp_t)
                while pending:
                    pjc, ppt, pp_t = pending.pop(0)
                    emit_pv(pjc, ppt, pp_t)

                # normalize: out_h = po[:, :D] * (1 / po[:, D]); since wO
                # is folded into the V projection these ARE output rows.
                for ib in range(4):
                    ic = qo * 4 + ib
                    rs_t = rsp.tile([P, 1], f32, tag="rs")
                    nc.vector.reciprocal(out=rs_t[:], in_=po[ib][:, D:D + 1])
                    if h == 0:
                        ret_t = retp.tile([P, D], f32, tag="ret")
                        nc.scalar.activation(
                            out=ret_t[:], in_=po[ib][:, 0:D],
                            func=mybir.ActivationFunctionType.Copy,
                            scale=rs_t[:])
                        ret0[ic] = ret_t
                    else:
                        ob = outp.tile([P, D], f32, tag="out")
                        nc.vector.scalar_tensor_tensor(
                            out=ob[:], in0=po[ib][:, 0:D], scalar=rs_t[:],
                            in1=ret0[ic][:], op0=mybir.AluOpType.mult,
                            op1=mybir.AluOpType.add)
                        nc.sync.dma_start(out=out_d[ic * P:(ic + 1) * P, :],
                                          in_=ob[:])

    nc.compile()
    return nc


def _make_mask():
    # mask[r, t*SG + c] = 1 if (t*P + r) <= c else 0  (keep j <= i)
    r = np.arange(P)[:, None]
    c = np.arange(SG)[None, :]
    blocks = [((t * P + r) <= c).astype(np.float32) for t in range(4)]
    return np.concatenate(blocks, axis=1)


def _in_maps(inputs, wQ, wK, wV, wO):
    mask = _make_mask()
    maps = []
    for core in range(N_CORES):
        b = core // 4
        h0 = 2 * (core % 4)
        hs = [h0, h0 + 1]
        xt = np.ascontiguousarray(inputs[b].T)  # [D, S]
        wq = np.stack([wQ[h].reshape(DC, P, D) for h in hs])
        wk = np.stack([wK[h].reshape(DC, P, D) for h in hs])
        wvo = np.stack([
            (wV[h].astype(np.float64)
             @ wO[h * D:(h + 1) * D, :].astype(np.float64))
            .astype(np.float32).reshape(DC, P, D) for h in hs])
        maps.append({
            "xt": xt,
            "wq": np.ascontiguousarray(wq),
            "wk": np.ascontiguousarray(wk),
            "wvo": np.ascontiguousarray(wvo),
            "mask": mask,
        })
    return maps


def _run(inputs, wQ, wK, wV, wO, trace=False, tmpdir=None):
    from concourse.bass_utils import run_bass_kernel_spmd

    if "nc" not in _STATE:
        _STATE["nc"] = _build_nc()
    maps = _in_maps(inputs, wQ, wK, wV, wO)
    res = run_bass_kernel_spmd(_STATE["nc"], maps, list(range(N_CORES)),
                               trace=trace, tmpdir=tmpdir)
    out = np.zeros((B, S, D), dtype=np.float32)
    for core in range(N_CORES):
        out[core // 4] += res.results[core]["out"]
    return out, res


def kernel(inputs, timestamp, wQ, wK, wV, wO, theta):
    inputs = np.asarray(inputs, dtype=np.float32)
    out, _ = _run(inputs, np.asarray(wQ, np.float32),
                  np.asarray(wK, np.float32), np.asarray(wV, np.float32),
                  np.asarray(wO, np.float32))
    return out


def kernel_profiled(inputs, timestamp, wQ, wK, wV, wO, theta, tmpdir=None):
    inputs = np.asarray(inputs, dtype=np.float32)
    out, res = _run(inputs, np.asarray(wQ, np.float32),
                    np.asarray(wK, np.float32), np.asarray(wV, np.float32),
                    np.asarray(wO, np.float32), trace=True, tmpdir=tmpdir)
    return out, res


# revision 34
# speedup vs baseline: 3.0122x; 3.0122x over previous
"""Self-contained Trainium2 Bass kernel for causal multi-head attention.

The reference computes softmax(QK^T/sqrt(D)) * tril, renormalized — which is
exactly a causal softmax (the full-row max/normalizer cancel) — followed by
P@V, head concat and an output projection. The time-decay branch is dead code.

Sharding: 16 (batch, head) pairs across 8 cores, 2 heads of one batch per
core. Each core computes partial_out = sum_h (attn_h @ wO_h) for its 2 heads
([2048, 256]); the host adds the 4 partials per batch element.

Per-core layout (all SBUF, fp32):
  XT [d, s]            model-dim-major activations
  QT/KT [e, s]         projections computed transposed so scores come out as
                       S^T[j, i] = KT.T @ QT with no softmax-side transposes
  V [s, e | 1 | 0]     ones column appended -> PV matmul emits row sums free
  PV accumulation in PSUM over causal j chunks; normalization (1/rowsum) is
  folded into the PSUM->SBUF copy; PE transposes flip ret [i,e] -> [e,i] for
  the head-sliced output projection.
"""

from contextlib import ExitStack

import numpy as np

B, S, D, H = 2, 2048, 256, 8
N_CORES = 8
P = 128          # partition size
SG = 512         # query group (i) width
NSG = S // SG    # 4 query groups
NJC = S // P     # 16 key chunks
EC = D // P      # 2 chunks along the head dim e
DC = D // P      # 2 chunks along the model dim d
VW = 258         # V tile width: 256 e cols + ones col + zero pad (even for f32r)

# Matmul operand dtype knob: "f32" (exact, 4 cyc/row) or "f32r" (fp32 with
# 12-bit mantissa, 1 cyc/row — 2.7x faster end-to-end, ~2e-4 rel err).
import os as _os
MM_DT = _os.environ.get("MM_DT", "f32r")

_STATE = {}


def _build_nc():
    import concourse.tile as tile
    from concourse import bacc, mybir

    f32 = mybir.dt.float32
    f32r = mybir.dt.float32r
    attn_dt = f32r if MM_DT == "f32r" else f32
    proj_dt = attn_dt

    def cast(ap, kind):
        return ap

    nc = bacc.Bacc("TRN2", target_bir_lowering=False, debug=False,
                   num_devices=N_CORES)

    xt_d = nc.dram_tensor("xt", [D, S], f32, kind="ExternalInput")
    wa_d = nc.dram_tensor("wa", [2, DC, P, D], f32, kind="ExternalInput")
    wvo_d = nc.dram_tensor("wvo", [2, DC, P, D], f32, kind="ExternalInput")
    mask_d = nc.dram_tensor("mask", [P, 4 * SG], f32, kind="ExternalInput")
    out_d = nc.dram_tensor("out", [S, D], f32, kind="ExternalOutput")

    with tile.TileContext(nc) as tc, ExitStack() as ctx:
        pool = lambda name, bufs, **kw: ctx.enter_context(
            tc.tile_pool(name=name, bufs=bufs, **kw))
        consts = pool("consts", 1)
        xtp = pool("xt", 2)
        stg = pool("stg", 2)
        wts = pool("wts", 4)
        qkp = pool("qk", 4)
        vp = pool("v", 2 * NJC)
        ptp = pool("pt", 4)
        rsp = pool("rs", 8)
        retp = pool("ret", NJC + 8)
        outp = pool("outsb", 4)
        ps_big = pool("ps_big", 4, space="PSUM")
        ps_acc = pool("ps_acc", 4, space="PSUM")

        mask_sb = consts.tile([P, 4 * SG], f32)
        nc.sync.dma_start(out=mask_sb[:], in_=mask_d[:])

        xt_sb = []
        for dc in range(DC):
            t = xtp.tile([P, S], proj_dt, tag="xt")
            if proj_dt == f32:
                nc.sync.dma_start(out=t[:], in_=xt_d[dc * P:(dc + 1) * P, :])
            else:
                raw = stg.tile([P, S], f32, tag="stg", name="xtraw")
                nc.sync.dma_start(out=raw[:], in_=xt_d[dc * P:(dc + 1) * P, :])
                nc.vector.tensor_copy(out=t[:], in_=raw[:])
            xt_sb.append(t)

        w_sb = {}
        for name, dram in (("wa", wa_d), ("wvo", wvo_d)):
            for h in range(2):
                t = wts.tile([P, DC, D], proj_dt, tag="w")
                if proj_dt == f32:
                    for dc in range(DC):
                        nc.sync.dma_start(out=t[:, dc, :], in_=dram[h, dc])
                else:
                    raw = stg.tile([P, DC, D], f32, tag="wstg", name="wraw")
                    for dc in range(DC):
                        nc.sync.dma_start(out=raw[:, dc, :], in_=dram[h, dc])
                    nc.vector.tensor_copy(out=t[:], in_=raw[:])
                w_sb[name, h] = t

        ret0 = {}  # ic -> head0 normalized output chunk [P, D]

        for h in range(2):
            # --- projections: KAT = (wK wQ^T) @ X^T and V' = X (wV wO_h) ---
            # scores come from X A X^T (A = wQ wK^T folded on the host), so
            # only ONE score-side projection is needed; X^T itself is the
            # moving operand of the score matmuls.
            kt = [qkp.tile([P, S], attn_dt, tag="qk", name="kt")
                  for _ in range(EC)]
            w = w_sb["wa", h]
            for ec in range(EC):
                for sg in range(NSG):
                    ps = ps_big.tile([P, SG], f32, tag="big")
                    for dc in range(DC):
                        nc.tensor.matmul(
                            ps[:],
                            cast(w[:, dc, ec * P:(ec + 1) * P], "proj"),
                            cast(xt_sb[dc][:, sg * SG:(sg + 1) * SG],
                                 "proj"),
                            start=(dc == 0), stop=(dc == DC - 1))
                    nc.vector.tensor_copy(
                        out=kt[ec][:, sg * SG:(sg + 1) * SG], in_=ps[:])
            v_sb = []
            wv = w_sb["wvo", h]
            for jc in range(NJC):
                ps = ps_acc.tile([P, VW], f32, tag="acc")
                for dc in range(DC):
                    nc.tensor.matmul(
                        ps[:, 0:D],
                        cast(xt_sb[dc][:, jc * P:(jc + 1) * P], "proj"),
                        cast(wv[:, dc, :], "proj"),
                        start=(dc == 0), stop=(dc == DC - 1))
                vt = vp.tile([P, VW], attn_dt, tag="v")
                nc.gpsimd.memset(vt[:, D:D + 1].bitcast(f32), 1.0)
                nc.gpsimd.memset(vt[:, D + 1:VW].bitcast(f32), 0.0)
                nc.vector.tensor_copy(out=vt[:, 0:D], in_=ps[:, 0:D])
                v_sb.append(vt)

            # --- attention: S^T chunks -> exp -> (mask) -> PV accumulate ---
            # Diagonal chunks are trimmed: chunk t only needs i >= t*128, so
            # compute columns [c0, SG) with c0 = min(t,2)*128 (c0 capped so
            # the f32r moving dim stays >= 256) and skip PV sub-blocks ib < t.
            for qo in range(NSG):
                njc = (qo + 1) * 4
                po = [ps_acc.tile([P, VW], f32, tag="acc", name="po") for _ in range(4)]

                def emit_pv(pjc, ppt, t):
                    for ib in range(max(t, 0), 4):
                        nc.tensor.matmul(
                            po[ib][:],
                            cast(ppt[:, ib * P:(ib + 1) * P], "attn"),
                            cast(v_sb[pjc][:], "attn"),
                            start=(pjc == 0), stop=(ib == t))

                pending = []
                for jc in range(njc):
                    t = jc - qo * 4
                    c0 = min(t, 2) * P if t > 0 else 0
                    ps = ps_big.tile([P, SG], f32, tag="big")
                    for ec in range(EC):
                        nc.tensor.matmul(
                            ps[:, c0:SG],
                            cast(kt[ec][:, jc * P:(jc + 1) * P], "attn"),
                            cast(xt_sb[ec][:, qo * SG + c0:(qo + 1) * SG],
                                 "attn"),
                            start=(ec == 0), stop=(ec == EC - 1))
                    pt = ptp.tile([P, SG], f32, tag="pt")
                    nc.scalar.activation(
                        out=pt[:, c0:SG], in_=ps[:, c0:SG],
                        func=mybir.ActivationFunctionType.Exp, scale=1.0 / 16.0)
                    if attn_dt == f32:
                        if t >= 0:
                            nc.vector.tensor_mul(
                                pt[:, c0:SG], pt[:, c0:SG],
                                mask_sb[:, t * SG + c0:(t + 1) * SG])
                        pv_src = pt
                    else:
                        # ACT can't emit f32r; round on DVE (the diagonal
                        # chunks fuse the causal mask into the rounding mul).
                        ptr = ptp.tile([P, SG], attn_dt, tag="ptr", name="ptr")
                        if t >= 0:
                            nc.vector.tensor_mul(
                                ptr[:, c0:SG], pt[:, c0:SG],
                                mask_sb[:, t * SG + c0:(t + 1) * SG])
                        else:
                            nc.vector.tensor_copy(out=ptr[:], in_=pt[:])
                        pv_src = ptr
                    pending.append((jc, pv_src, t))
                    if len(pending) > 3:
                        pjc, ppt, pp_t = pending.pop(0)
                        emit_pv(pjc, ppt, pp_t)
                while pending:
                    pjc, ppt, pp_t = pending.pop(0)
                    emit_pv(pjc, ppt, pp_t)

                # normalize: out_h = po[:, :D] * (1 / po[:, D]); since wO
                # is folded into the V projection these ARE output rows.
                for ib in range(4):
                    ic = qo * 4 + ib
                    rs_t = rsp.tile([P, 1], f32, tag="rs")
                    nc.vector.reciprocal(out=rs_t[:], in_=po[ib][:, D:D + 1])
                    if h == 0:
                        ret_t = retp.tile([P, D], f32, tag="ret")
                        nc.scalar.activation(
                            out=ret_t[:], in_=po[ib][:, 0:D],
                            func=mybir.ActivationFunctionType.Copy,
                            scale=rs_t[:])
                        ret0[ic] = ret_t
                    else:
                        ob = outp.tile([P, D], f32, tag="out")
                        nc.vector.scalar_tensor_tensor(
                            out=ob[:], in0=po[ib][:, 0:D], scalar=rs_t[:],
                            in1=ret0[ic][:], op0=mybir.AluOpType.mult,
                            op1=mybir.AluOpType.add)
                        nc.sync.dma_start(out=out_d[ic * P:(ic + 1) * P, :],
                                          in_=ob[:])

    nc.compile()
    return nc


def _make_mask():
    # mask[r, t*SG + c] = 1 if (t*P + r) <= c else 0  (keep j <= i)
    r = np.arange(P)[:, None]
    c = np.arange(SG)[None, :]
    blocks = [((t * P + r) <= c).astype(np.float32) for t in range(4)]
    return np.concatenate(blocks, axis=1)


def _in_maps(inputs, wQ, wK, wV, wO):
    mask = _make_mask()
    maps = []
    for core in range(N_CORES):
        b = core // 4
        h0 = 2 * (core % 4)
        hs = [h0, h0 + 1]
        xt = np.ascontiguousarray(inputs[b].T)  # [D, S]
        wa = np.stack([
            (wK[h].astype(np.float64) @ wQ[h].astype(np.float64).T)
            .astype(np.float32).reshape(DC, P, D) for h in hs])
        wvo = np.stack([
            (wV[h].astype(np.float64)
             @ wO[h * D:(h + 1) * D, :].astype(np.float64))
            .astype(np.float32).reshape(DC, P, D) for h in hs])
        maps.append({
            "xt": xt,
            "wa": np.ascontiguousarray(wa),
            "wvo": np.ascontiguousarray(wvo),
            "mask": mask,
        })
    return maps


def _run(inputs, wQ, wK, wV, wO, trace=False, tmpdir=None):
    import time

    from concourse.bass_utils import run_bass_kernel_spmd

    if "nc" not in _STATE:
        _STATE["nc"] = _build_nc()
    maps = _in_maps(inputs, wQ, wK, wV, wO)
    res = None
    for attempt in range(3):
        try:
            res = run_bass_kernel_spmd(_STATE["nc"], maps,
                                       list(range(N_CORES)),
                                       trace=trace, tmpdir=tmpdir)
            break
        except Exception:
            # transient NRT device faults have been observed; retry
            if attempt == 2:
                raise
            time.sleep(2.0)
    out = np.zeros((B, S, D), dtype=np.float32)
    for core in range(N_CORES):
        out[core // 4] += res.results[core]["out"]
    return out, res


def kernel(inputs, timestamp, wQ, wK, wV, wO, theta):
    inputs = np.asarray(inputs, dtype=np.float32)
    out, _ = _run(inputs, np.asarray(wQ, np.float32),
                  np.asarray(wK, np.float32), np.asarray(wV, np.float32),
                  np.asarray(wO, np.float32))
    return out


def kernel_profiled(inputs, timestamp, wQ, wK, wV, wO, theta, tmpdir=None):
    inputs = np.asarray(inputs, dtype=np.float32)
    out, res = _run(inputs, np.asarray(wQ, np.float32),
                    np.asarray(wK, np.float32), np.asarray(wV, np.float32),
                    np.asarray(wO, np.float32), trace=True, tmpdir=tmpdir)
    return out, res
